# revision 1
# baseline (speedup 1.0000x reference)
"""Trainium2 Bass kernel for nn_CustomQuaternionLoss.

Computes mean over B samples of mean-over-3-components HuberLoss(delta=1)
of the rotation vector of  R(pred_quat) * R(true_quat)^-1.

Mathematical reformulation (verified vs reference, rel err ~2e-7):
  d = p (x) conj(t)   (unnormalized quaternion product; norms divide out)
  Every downstream quantity is even in each component of d, so the
  w>=0 canonicalization is free (|rw|) and component signs are dropped.
  angle = 2*atan2(|v|, |w|)  = 4*atan(|v| / (sqrt(|v|^2+w^2) + |w|))
      (tangent half-angle form keeps the Arctan spline argument in [0,1];
       the ACT Arctan table is only valid on [-pi/2, pi/2])
  rotvec_i = angle * v_i / |v|
  huber(x) = 0.5*x^2 - 0.5*relu(|x|-1)^2
  sum_i huber(rotvec_i) = 0.5*angle^2 - 0.5*sum_i relu(angle*|v_i|/|v| - 1)^2
  (the small-angle Taylor branch of the reference is unreachable for the
   randn inputs: min angle ~0.012 >> 1e-3)

Sharding: batch dim split evenly across 8 cores; each core reduces its
524288 samples to a per-partition [128,2] partial-sum pair; host combines.
batch_X is unused by the reference and is never touched.

Built on Bacc (not raw Bass): walrus only accepts one sync wait per
instruction, and Bacc's generate_event_semaphores pass splits multi-wait
instructions into EventSemaphore + op.
"""

import numpy as np

import concourse.bacc as bacc
import concourse.mybir as mybir
import concourse.tile as tile
from concourse.bass_utils import run_bass_kernel_spmd

B = 4194304
NCORES = 8
S = B // NCORES  # samples per core
P = 128
F = 1024  # samples per partition per tile
NT = S // (P * F)  # tiles per core (4)

F32 = mybir.dt.float32
BF16 = mybir.dt.bfloat16
AF = mybir.ActivationFunctionType
ALU = mybir.AluOpType

_SQ8 = float(np.sqrt(8.0))  # 0.5*angle^2 = 0.5*(4*at)^2 = (sqrt(8)*at)^2


def _build_nc():
    nc = bacc.Bacc(
        "TRN2", target_bir_lowering=False, debug=False, num_devices=NCORES
    )
    tq = nc.dram_tensor("tq", [S, 4], F32, kind="ExternalInput")
    pq = nc.dram_tensor("pq", [S, 4], F32, kind="ExternalInput")
    out = nc.dram_tensor("out", [P, 2], F32, kind="ExternalOutput")

    tqv = tq.ap().rearrange("(n p f) c -> n p (f c)", p=P, f=F)
    pqv = pq.ap().rearrange("(n p f) c -> n p (f c)", p=P, f=F)

    with tile.TileContext(nc) as tc:
        with (
            tc.tile_pool(name="io", bufs=2) as io_pool,
            tc.tile_pool(name="wk", bufs=1) as wk,
            tc.tile_pool(name="acc", bufs=1) as acc_pool,
        ):
            t1cols = acc_pool.tile([P, NT], F32, tag="t1c", name="t1cols")
            hacols = acc_pool.tile([P, 3 * NT], F32, tag="hac", name="hacols")
            negone = acc_pool.tile([P, 1], F32, tag="negone", name="negone")
            nc.vector.memset(negone[:], -1.0)

            def WT(tag, bufs=None, dt=F32):
                return wk.tile([P, F], dt, tag=tag, name=tag, bufs=bufs)

            def WT4(tag):
                return wk.tile([P, 4 * F], BF16, tag=tag, name=tag, bufs=2)

            for i in range(NT):
                tt = io_pool.tile([P, 4 * F], F32, tag="tq", name="tqt")
                pt = io_pool.tile([P, 4 * F], F32, tag="pq", name="pqt")
                nc.sync.dma_start(tt[:], tqv[i])
                nc.sync.dma_start(pt[:], pqv[i])

                tv = tt[:].rearrange("p (f c) -> p c f", c=4)
                pv = pt[:].rearrange("p (f c) -> p c f", c=4)
                tx, ty, tz, tw = (tv[:, c, :] for c in range(4))
                px, py, pz, pw = (pv[:, c, :] for c in range(4))

                # ---- stage A: d = p * conj(t) (unnormalized) ----
                # rx = (tw*px - pw*tx) - (py*tz - pz*ty)
                # ry = (tw*py - pw*ty) - (pz*tx - px*tz)
                # rz = (tw*pz - pw*tz) - (px*ty - py*tx)
                # rw = (pw*tw + px*tx) + (py*ty + pz*tz)
                # DVE/GPSIMD split tuned for balance (GPSIMD elementwise is
                # roughly half DVE throughput but runs concurrently).
                # All four chains share wide [P, 4F] product tiles
                # (col blocks: rx|ry|rz|rw), so the 12 combine adds fuse
                # into 3 double-wide bf16 ops in 2x mode.  rw's all-plus
                # chain is made subtract-compatible by negating two of its
                # products via the free STT scalar pre-op:
                #   rw = (D - (-A)) - ((-B) - C)
                prods1 = [(tw, px), (pw, tx), (tw, py), (pw, ty),
                          (tw, pz), (pw, tz), (pw, tw), (px, tx)]
                prods2 = [(py, tz), (pz, ty), (pz, tx), (px, tz),
                          (px, ty), (py, tx), (py, ty), (pz, tz)]
                neg1 = {7}   # -A  (MB col block 3)
                neg2 = {6}   # -B  (MA2 col block 3)
                MA = WT4("ma4")
                MB = WT4("mb4")
                for k in range(0, 8, 2):
                    a, b = prods1[k], prods1[k + 1]
                    j = k // 2
                    sl = slice(j * F, (j + 1) * F)
                    nc.vector.tensor_tensor(MA[:, sl], a[0], a[1], ALU.mult)
                    if k + 1 in neg1:
                        nc.vector.scalar_tensor_tensor(
                            MB[:, sl], b[0], -1.0, b[1], ALU.mult, ALU.mult
                        )
                    else:
                        nc.vector.tensor_tensor(MB[:, sl], b[0], b[1], ALU.mult)
                nc.vector.tensor_tensor(MA[:], MA[:], MB[:], ALU.subtract)
                MA2 = WT4("ma4")
                MB2 = WT4("mb4")
                for k in range(0, 8, 2):
                    a, b = prods2[k], prods2[k + 1]
                    j = k // 2
                    sl = slice(j * F, (j + 1) * F)
                    if k in neg2:
                        nc.vector.scalar_tensor_tensor(
                            MA2[:, sl], a[0], -1.0, a[1], ALU.mult, ALU.mult
                        )
                    else:
                        nc.vector.tensor_tensor(MA2[:, sl], a[0], a[1], ALU.mult)
                    nc.vector.tensor_tensor(MB2[:, sl], b[0], b[1], ALU.mult)
                nc.vector.tensor_tensor(MA2[:], MA2[:], MB2[:], ALU.subtract)
                RC = WT4("rc4")
                nc.vector.tensor_tensor(RC[:], MA[:], MA2[:], ALU.subtract)
                rx = RC[:, 0:F]
                ry = RC[:, F : 2 * F]
                rz = RC[:, 2 * F : 3 * F]
                rw = RC[:, 3 * F : 4 * F]

                # ---- stage B ----
                z1 = WT("z1", bufs=2)
                z2 = WT("z2", bufs=2)
                nc.scalar.activation(z1[:], rx[:], AF.Square)
                nc.scalar.activation(z2[:], ry[:], AF.Square)
                # q-chain tile: vn2 -> inv_vn -> g   (in place)
                q = WT("q", bufs=2)
                nc.vector.tensor_tensor(q[:], z1[:], z2[:], ALU.add)
                z3 = WT("z1", bufs=2)
                nc.scalar.activation(z3[:], rz[:], AF.Square)
                nc.vector.tensor_tensor(q[:], q[:], z3[:], ALU.add)  # q = vn2
                # w-chain tile: rw2 -> dn2 -> dn -> den -> invden -> r -> at
                w = WT("w", bufs=2)
                nc.scalar.activation(w[:], rw[:], AF.Square)  # rw^2
                nc.vector.tensor_tensor(w[:], w[:], q[:], ALU.add)  # dn2
                nc.scalar.activation(w[:], w[:], AF.Sqrt)  # dn
                # den = |rw| + dn
                wab = WT("wab", bufs=2)
                nc.scalar.activation(wab[:], rw[:], AF.Abs)
                nc.vector.tensor_tensor(w[:], wab[:], w[:], ALU.add)
                winv = WT("winv", bufs=2)
                nc.vector.reciprocal_approx_fast(winv[:], w[:])  # 1/den
                vn = WT("vn", bufs=2)
                nc.scalar.activation(vn[:], q[:], AF.Sqrt)  # vn
                nc.vector.tensor_tensor(winv[:], vn[:], winv[:], ALU.mult)  # r
                nc.scalar.activation(winv[:], winv[:], AF.Arctan)  # at

                junk = WT("junk", bufs=2)
                nc.scalar.activation(
                    junk[:], winv[:], AF.Square, scale=_SQ8,
                    accum_out=t1cols[:, i : i + 1],
                )

                nc.vector.reciprocal_approx_fast(q[:], vn[:])  # q = 1/vn
                # g = 4*at/vn   (kept fp32: a bf16 u-path costs 3 decades of
                # accuracy margin for ~1% speed)
                nc.vector.scalar_tensor_tensor(
                    q[:], winv[:], 4.0, q[:], ALU.mult, ALU.mult
                )

                for k, rv in enumerate((rx, ry, rz)):
                    av = WT("av", bufs=2)
                    nc.scalar.activation(av[:], rv[:], AF.Abs)
                    u = WT("u", bufs=2)
                    nc.vector.tensor_tensor(u[:], av[:], q[:], ALU.mult)
                    # h = relu(u - 1), then accumulate h^2 along the free dim
                    hr = WT("hr", bufs=2)
                    nc.scalar.activation(hr[:], u[:], AF.Relu, bias=negone[:])
                    junk2 = WT("junk", bufs=2)
                    nc.scalar.activation(
                        junk2[:], hr[:], AF.Square,
                        accum_out=hacols[:, 3 * i + k : 3 * i + k + 1],
                    )

            res = acc_pool.tile([P, 2], F32, tag="res", name="res")
            nc.vector.tensor_reduce(
                res[:, 0:1], t1cols[:], mybir.AxisListType.X, ALU.add
            )
            nc.vector.tensor_reduce(
                res[:, 1:2], hacols[:], mybir.AxisListType.X, ALU.add
            )
            nc.gpsimd.dma_start(out.ap(), res[:])

    nc.compile()
    return nc


_CACHED_NC = None


def _get_nc():
    global _CACHED_NC
    if _CACHED_NC is None:
        _CACHED_NC = _build_nc()
    return _CACHED_NC


def run_sharded(tq_full, pq_full, **run_kwargs):
    """Run the SPMD kernel; returns BassKernelResults."""
    nc = _get_nc()
    in_maps = []
    for c in range(NCORES):
        sl = slice(c * S, (c + 1) * S)
        in_maps.append(
            {
                "tq": np.ascontiguousarray(tq_full[sl]),
                "pq": np.ascontiguousarray(pq_full[sl]),
            }
        )
    return run_bass_kernel_spmd(nc, in_maps, list(range(NCORES)), **run_kwargs)


def kernel(
    true_quaternions: np.ndarray,
    predicted_quaternion: np.ndarray,
    batch_X: np.ndarray = None,
    **_ignored,
) -> np.ndarray:
    res = run_sharded(true_quaternions, predicted_quaternion)
    total = 0.0
    for core in res.results:
        v = core["out"].astype(np.float64)
        total += v[:, 0].sum() - 0.5 * v[:, 1].sum()
    loss = total / (3.0 * B)
    return np.float32(loss)



# revision 3
# speedup vs baseline: 1.2098x; 1.2098x over previous
"""Trainium2 Bass kernel for nn_CustomQuaternionLoss.

Computes mean over B samples of mean-over-3-components HuberLoss(delta=1)
of the rotation vector of  R(pred_quat) * R(true_quat)^-1.

Mathematical reformulation (verified vs reference, rel err ~2e-7):
  d = p (x) conj(t)   (unnormalized quaternion product; norms divide out)
  Every downstream quantity is even in each component of d, so the
  w>=0 canonicalization is free (|rw|) and component signs are dropped.
  angle = 2*atan2(|v|, |w|)  = 4*atan(|v| / (sqrt(|v|^2+w^2) + |w|))
      (tangent half-angle form keeps the Arctan spline argument in [0,1];
       the ACT Arctan table is only valid on [-pi/2, pi/2])
  rotvec_i = angle * v_i / |v|
  huber(x) = 0.5*x^2 - 0.5*relu(|x|-1)^2
  sum_i huber(rotvec_i) = 0.5*angle^2 - 0.5*sum_i relu(angle*|v_i|/|v| - 1)^2
  (the small-angle Taylor branch of the reference is unreachable for the
   randn inputs: min angle ~0.012 >> 1e-3)

Sharding: batch dim split evenly across 8 cores; each core reduces its
524288 samples to a per-partition [128,2] partial-sum pair; host combines.
batch_X is unused by the reference and is never touched.

Built on Bacc (not raw Bass): walrus only accepts one sync wait per
instruction, and Bacc's generate_event_semaphores pass splits multi-wait
instructions into EventSemaphore + op.
"""

import numpy as np

import concourse.bacc as bacc
import concourse.mybir as mybir
import concourse.tile as tile
from concourse.bass_utils import run_bass_kernel_spmd

B = 4194304
NCORES = 8
S = B // NCORES  # samples per core
P = 128
F = 1024  # samples per partition per tile
NT = S // (P * F)  # tiles per core (4)

F32 = mybir.dt.float32
BF16 = mybir.dt.bfloat16
AF = mybir.ActivationFunctionType
ALU = mybir.AluOpType

_SQ8 = float(np.sqrt(8.0))  # 0.5*angle^2 = 0.5*(4*at)^2 = (sqrt(8)*at)^2


def _build_nc():
    nc = bacc.Bacc(
        "TRN2", target_bir_lowering=False, debug=False, num_devices=NCORES
    )
    tq = nc.dram_tensor("tq", [S, 4], F32, kind="ExternalInput")
    pq = nc.dram_tensor("pq", [S, 4], F32, kind="ExternalInput")
    out = nc.dram_tensor("out", [P, 2], F32, kind="ExternalOutput")

    tqv = tq.ap().rearrange("(n p f) c -> n p (f c)", p=P, f=F)
    pqv = pq.ap().rearrange("(n p f) c -> n p (f c)", p=P, f=F)

    with tile.TileContext(nc) as tc:
        with (
            tc.tile_pool(name="io", bufs=2) as io_pool,
            tc.tile_pool(name="wk", bufs=1) as wk,
            tc.tile_pool(name="acc", bufs=1) as acc_pool,
        ):
            t1cols = acc_pool.tile([P, NT], F32, tag="t1c", name="t1cols")
            hacols = acc_pool.tile([P, 3 * NT], F32, tag="hac", name="hacols")
            negone = acc_pool.tile([P, 1], F32, tag="negone", name="negone")
            nc.vector.memset(negone[:], -1.0)

            def WT(tag, bufs=None, dt=F32):
                return wk.tile([P, F], dt, tag=tag, name=tag, bufs=bufs)

            def WT4(tag):
                return wk.tile([P, 4 * F], BF16, tag=tag, name=tag, bufs=2)

            for i in range(NT):
                tt = io_pool.tile([P, 4 * F], F32, tag="tq", name="tqt")
                pt = io_pool.tile([P, 4 * F], F32, tag="pq", name="pqt")
                nc.sync.dma_start(tt[:], tqv[i])
                nc.sync.dma_start(pt[:], pqv[i])

                # bf16 copies: strided product reads then run at 2B width
                # (measured 1211ns vs 1952ns per [P,1024] stride-4 mult)
                tb = wk.tile([P, 4 * F], BF16, tag="tb", name="tb", bufs=1)
                pb = wk.tile([P, 4 * F], BF16, tag="pb", name="pb", bufs=1)
                nc.vector.tensor_copy(tb[:], tt[:])
                nc.scalar.activation(pb[:], pt[:], AF.Copy)

                tv = tb[:].rearrange("p (f c) -> p c f", c=4)
                pv = pb[:].rearrange("p (f c) -> p c f", c=4)
                tx, ty, tz, tw = (tv[:, c, :] for c in range(4))
                px, py, pz, pw = (pv[:, c, :] for c in range(4))

                # ---- stage A: d = p * conj(t) (unnormalized) ----
                # rx = (tw*px - pw*tx) - (py*tz - pz*ty)
                # ry = (tw*py - pw*ty) - (pz*tx - px*tz)
                # rz = (tw*pz - pw*tz) - (px*ty - py*tx)
                # rw = (pw*tw + px*tx) + (py*ty + pz*tz)
                # DVE/GPSIMD split tuned for balance (GPSIMD elementwise is
                # roughly half DVE throughput but runs concurrently).
                # All four chains share wide [P, 4F] product tiles
                # (col blocks: rx|ry|rz|rw), so the 12 combine adds fuse
                # into 3 double-wide bf16 ops in 2x mode.  rw's all-plus
                # chain is made subtract-compatible by negating two of its
                # products via the free STT scalar pre-op:
                #   rw = (D - (-A)) - ((-B) - C)
                prods1 = [(tw, px), (pw, tx), (tw, py), (pw, ty),
                          (tw, pz), (pw, tz), (pw, tw), (px, tx)]
                prods2 = [(py, tz), (pz, ty), (pz, tx), (px, tz),
                          (px, ty), (py, tx), (py, ty), (pz, tz)]
                neg1 = {7}   # -A  (MB col block 3)
                neg2 = {6}   # -B  (MA2 col block 3)
                MA = WT4("ma4")
                MB = WT4("mb4")
                for k in range(0, 8, 2):
                    a, b = prods1[k], prods1[k + 1]
                    j = k // 2
                    sl = slice(j * F, (j + 1) * F)
                    nc.vector.tensor_tensor(MA[:, sl], a[0], a[1], ALU.mult)
                    if k + 1 in neg1:
                        nc.vector.scalar_tensor_tensor(
                            MB[:, sl], b[0], -1.0, b[1], ALU.mult, ALU.mult
                        )
                    else:
                        nc.vector.tensor_tensor(MB[:, sl], b[0], b[1], ALU.mult)
                nc.vector.tensor_tensor(MA[:], MA[:], MB[:], ALU.subtract)
                MA2 = WT4("ma4")
                MB2 = WT4("mb4")
                for k in range(0, 8, 2):
                    a, b = prods2[k], prods2[k + 1]
                    j = k // 2
                    sl = slice(j * F, (j + 1) * F)
                    if k in neg2:
                        nc.vector.scalar_tensor_tensor(
                            MA2[:, sl], a[0], -1.0, a[1], ALU.mult, ALU.mult
                        )
                    else:
                        nc.vector.tensor_tensor(MA2[:, sl], a[0], a[1], ALU.mult)
                    nc.vector.tensor_tensor(MB2[:, sl], b[0], b[1], ALU.mult)
                nc.vector.tensor_tensor(MA2[:], MA2[:], MB2[:], ALU.subtract)
                RC = WT4("rc4")
                nc.vector.tensor_tensor(RC[:], MA[:], MA2[:], ALU.subtract)
                rx = RC[:, 0:F]
                ry = RC[:, F : 2 * F]
                rz = RC[:, 2 * F : 3 * F]
                rw = RC[:, 3 * F : 4 * F]

                # ---- stage B ----
                z1 = WT("z1", bufs=2, dt=BF16)
                z2 = WT("z2", bufs=2, dt=BF16)
                nc.scalar.activation(z1[:], rx[:], AF.Square)
                nc.scalar.activation(z2[:], ry[:], AF.Square)
                # q = vn2 (bf16: 621ns adds vs 1100 f32)
                q = WT("q", bufs=2, dt=BF16)
                nc.vector.tensor_tensor(q[:], z1[:], z2[:], ALU.add)
                z3 = WT("z1", bufs=2, dt=BF16)
                nc.scalar.activation(z3[:], rz[:], AF.Square)
                nc.vector.tensor_tensor(q[:], q[:], z3[:], ALU.add)  # q = vn2
                # dn2 chain in bf16, then f32 from Sqrt onward
                w2 = WT("w2", bufs=2, dt=BF16)
                nc.scalar.activation(w2[:], rw[:], AF.Square)  # rw^2
                nc.vector.tensor_tensor(w2[:], w2[:], q[:], ALU.add)  # dn2
                w = WT("w", bufs=2)
                nc.scalar.activation(w[:], w2[:], AF.Sqrt)  # dn (f32)
                # den = |rw| + dn
                wab = WT("wab", bufs=2)
                nc.scalar.activation(wab[:], rw[:], AF.Abs)
                nc.vector.tensor_tensor(w[:], wab[:], w[:], ALU.add)
                winv = WT("winv", bufs=2)
                nc.vector.reciprocal_approx_fast(winv[:], w[:])  # 1/den
                vn = WT("vn", bufs=2)
                nc.scalar.activation(vn[:], q[:], AF.Sqrt)  # vn (f32)
                nc.vector.tensor_tensor(winv[:], vn[:], winv[:], ALU.mult)  # r
                nc.scalar.activation(winv[:], winv[:], AF.Arctan)  # at

                junk = WT("junk", bufs=2, dt=BF16)
                nc.scalar.activation(
                    junk[:], winv[:], AF.Square, scale=_SQ8,
                    accum_out=t1cols[:, i : i + 1],
                )

                ivn = WT("ivn", bufs=2)
                nc.vector.reciprocal_approx_fast(ivn[:], vn[:])  # 1/vn
                # g = 4*at/vn (f32)
                nc.vector.scalar_tensor_tensor(
                    ivn[:], winv[:], 4.0, ivn[:], ALU.mult, ALU.mult
                )

                for k, rv in enumerate((rx, ry, rz)):
                    av = WT("av", bufs=2, dt=BF16)
                    nc.scalar.activation(av[:], rv[:], AF.Abs)
                    u = WT("u", bufs=2, dt=BF16)
                    nc.vector.tensor_tensor(u[:], av[:], ivn[:], ALU.mult)
                    # h = relu(u - 1), then accumulate h^2 along the free dim
                    hr = WT("hr", bufs=2, dt=BF16)
                    nc.scalar.activation(hr[:], u[:], AF.Relu, bias=negone[:])
                    junk2 = WT("junk", bufs=2, dt=BF16)
                    nc.scalar.activation(
                        junk2[:], hr[:], AF.Square,
                        accum_out=hacols[:, 3 * i + k : 3 * i + k + 1],
                    )

            res = acc_pool.tile([P, 2], F32, tag="res", name="res")
            nc.vector.tensor_reduce(
                res[:, 0:1], t1cols[:], mybir.AxisListType.X, ALU.add
            )
            nc.vector.tensor_reduce(
                res[:, 1:2], hacols[:], mybir.AxisListType.X, ALU.add
            )
            nc.gpsimd.dma_start(out.ap(), res[:])

    nc.compile()
    return nc


_CACHED_NC = None


def _get_nc():
    global _CACHED_NC
    if _CACHED_NC is None:
        _CACHED_NC = _build_nc()
    return _CACHED_NC


def run_sharded(tq_full, pq_full, **run_kwargs):
    """Run the SPMD kernel; returns BassKernelResults."""
    nc = _get_nc()
    in_maps = []
    for c in range(NCORES):
        sl = slice(c * S, (c + 1) * S)
        in_maps.append(
            {
                "tq": np.ascontiguousarray(tq_full[sl]),
                "pq": np.ascontiguousarray(pq_full[sl]),
            }
        )
    return run_bass_kernel_spmd(nc, in_maps, list(range(NCORES)), **run_kwargs)


def kernel(
    true_quaternions: np.ndarray,
    predicted_quaternion: np.ndarray,
    batch_X: np.ndarray = None,
    **_ignored,
) -> np.ndarray:
    res = run_sharded(true_quaternions, predicted_quaternion)
    total = 0.0
    for core in res.results:
        v = core["out"].astype(np.float64)
        total += v[:, 0].sum() - 0.5 * v[:, 1].sum()
    loss = total / (3.0 * B)
    return np.float32(loss)



# revision 4
# speedup vs baseline: 1.3426x; 1.1098x over previous
"""Trainium2 Bass kernel for nn_CustomQuaternionLoss.

Computes mean over B samples of mean-over-3-components HuberLoss(delta=1)
of the rotation vector of  R(pred_quat) * R(true_quat)^-1.

Mathematical reformulation (verified vs reference, rel err ~2e-7):
  d = p (x) conj(t)   (unnormalized quaternion product; norms divide out)
  Every downstream quantity is even in each component of d, so the
  w>=0 canonicalization is free (|rw|) and component signs are dropped.
  angle = 2*atan2(|v|, |w|)  = 4*atan(|v| / (sqrt(|v|^2+w^2) + |w|))
      (tangent half-angle form keeps the Arctan spline argument in [0,1];
       the ACT Arctan table is only valid on [-pi/2, pi/2])
  rotvec_i = angle * v_i / |v|
  huber(x) = 0.5*x^2 - 0.5*relu(|x|-1)^2
  sum_i huber(rotvec_i) = 0.5*angle^2 - 0.5*sum_i relu(angle*|v_i|/|v| - 1)^2
  (the small-angle Taylor branch of the reference is unreachable for the
   randn inputs: min angle ~0.012 >> 1e-3)

Sharding: batch dim split evenly across 8 cores; each core reduces its
524288 samples to a per-partition [128,2] partial-sum pair; host combines.
batch_X is unused by the reference and is never touched.

Built on Bacc (not raw Bass): walrus only accepts one sync wait per
instruction, and Bacc's generate_event_semaphores pass splits multi-wait
instructions into EventSemaphore + op.
"""

import numpy as np

import concourse.bacc as bacc
import concourse.mybir as mybir
import concourse.tile as tile
from concourse.bass_utils import run_bass_kernel_spmd

B = 4194304
NCORES = 8
S = B // NCORES  # samples per core
P = 128
F = 1024  # samples per partition per tile
NT = S // (P * F)  # tiles per core (4)

F32 = mybir.dt.float32
BF16 = mybir.dt.bfloat16
AF = mybir.ActivationFunctionType
ALU = mybir.AluOpType

_SQ8 = float(np.sqrt(8.0))  # 0.5*angle^2 = 0.5*(4*at)^2 = (sqrt(8)*at)^2


def _build_nc():
    nc = bacc.Bacc(
        "TRN2", target_bir_lowering=False, debug=False, num_devices=NCORES
    )
    tq = nc.dram_tensor("tq", [S, 4], F32, kind="ExternalInput")
    pq = nc.dram_tensor("pq", [S, 4], F32, kind="ExternalInput")
    out = nc.dram_tensor("out", [P, 2], F32, kind="ExternalOutput")

    tqv = tq.ap().rearrange("(n p f) c -> n p (f c)", p=P, f=F)
    pqv = pq.ap().rearrange("(n p f) c -> n p (f c)", p=P, f=F)

    with tile.TileContext(nc) as tc:
        with (
            tc.tile_pool(name="io", bufs=2) as io_pool,
            tc.tile_pool(name="wk", bufs=1) as wk,
            tc.tile_pool(name="acc", bufs=1) as acc_pool,
        ):
            t1cols = acc_pool.tile([P, NT], F32, tag="t1c", name="t1cols")
            hacols = acc_pool.tile([P, 3 * NT], F32, tag="hac", name="hacols")
            negone = acc_pool.tile([P, 1], F32, tag="negone", name="negone")
            nc.vector.memset(negone[:], -1.0)

            def WT(tag, bufs=None, dt=F32):
                return wk.tile([P, F], dt, tag=tag, name=tag, bufs=bufs)

            def WT4(tag):
                return wk.tile([P, 4 * F], BF16, tag=tag, name=tag, bufs=2)

            for i in range(NT):
                tt = io_pool.tile([P, 4 * F], F32, tag="tq", name="tqt")
                pt = io_pool.tile([P, 4 * F], F32, tag="pq", name="pqt")
                nc.sync.dma_start(tt[:], tqv[i])
                nc.sync.dma_start(pt[:], pqv[i])

                # bf16 copies: strided product reads then run at 2B width
                # (measured 1211ns vs 1952ns per [P,1024] stride-4 mult)
                tb = wk.tile([P, 4 * F], BF16, tag="tb", name="tb", bufs=1)
                pb = wk.tile([P, 4 * F], BF16, tag="pb", name="pb", bufs=1)
                nc.vector.tensor_copy(tb[:], tt[:])
                nc.scalar.activation(pb[:], pt[:], AF.Copy)

                tv = tb[:].rearrange("p (f c) -> p c f", c=4)
                pv = pb[:].rearrange("p (f c) -> p c f", c=4)
                tx, ty, tz, tw = (tv[:, c, :] for c in range(4))
                px, py, pz, pw = (pv[:, c, :] for c in range(4))

                # ---- stage A: d = p (x) conj(t), diagonal product scheme ----
                # Products grouped by component-diagonal (t_i*p_{i+k}) so one
                # TT op covers 2-4 products via packed [F,k] access patterns:
                #   rx = (t3p0-t0p3) - (t2p1-t1p2)
                #   ry = (t2p0-t0p2) + (t3p1-t1p3)
                #   rz = (t3p2-t2p3) - (t1p0-t0p1)
                #   rw = t0p0+t1p1+t2p2+t3p3
                # (per-component global signs are free: downstream is even)
                tv4 = tb[:].rearrange("p (f c) -> p f c", c=4)
                pv4 = pb[:].rearrange("p (f c) -> p f c", c=4)

                def DT(tag, w):
                    return wk.tile([P, w], BF16, tag=tag, name=tag, bufs=1)

                D0 = DT("d0", 4 * F)
                nc.vector.tensor_tensor(D0[:], tb[:], pb[:], ALU.mult)
                A1 = DT("a1", 3 * F)
                nc.vector.tensor_tensor(
                    A1[:].rearrange("p (f c) -> p f c", c=3),
                    tv4[:, :, 0:3], pv4[:, :, 1:4], ALU.mult)
                B1 = DT("b1", 3 * F)
                nc.vector.tensor_tensor(
                    B1[:].rearrange("p (f c) -> p f c", c=3),
                    tv4[:, :, 1:4], pv4[:, :, 0:3], ALU.mult)
                A2 = DT("a2", 2 * F)
                nc.vector.tensor_tensor(
                    A2[:].rearrange("p (f c) -> p f c", c=2),
                    tv4[:, :, 0:2], pv4[:, :, 2:4], ALU.mult)
                B2 = DT("b2", 2 * F)
                nc.vector.tensor_tensor(
                    B2[:].rearrange("p (f c) -> p f c", c=2),
                    tv4[:, :, 2:4], pv4[:, :, 0:2], ALU.mult)
                A3 = DT("a3", F)
                nc.vector.tensor_tensor(A3[:], tv4[:, :, 0], pv4[:, :, 3], ALU.mult)
                B3 = DT("b3", F)
                nc.vector.tensor_tensor(B3[:], tv4[:, :, 3], pv4[:, :, 0], ALU.mult)

                # combines (in-place diffs, then slot compactions into RC)
                nc.vector.tensor_tensor(B1[:], B1[:], A1[:], ALU.subtract)
                nc.vector.tensor_tensor(B2[:], B2[:], A2[:], ALU.subtract)
                nc.vector.tensor_tensor(B3[:], B3[:], A3[:], ALU.subtract)
                RC = WT4("rc4")
                cb1v = B1[:].rearrange("p (f c) -> p f c", c=3)
                cb2v = B2[:].rearrange("p (f c) -> p f c", c=2)
                nc.vector.tensor_tensor(
                    RC[:, 0:F], B3[:], cb1v[:, :, 1], ALU.subtract)
                nc.vector.tensor_tensor(
                    RC[:, F : 2 * F], cb2v[:, :, 0], cb2v[:, :, 1], ALU.add)
                nc.vector.tensor_tensor(
                    RC[:, 2 * F : 3 * F], cb1v[:, :, 2], cb1v[:, :, 0],
                    ALU.subtract)
                d0v = D0[:].rearrange("p (f c) -> p f c", c=4)
                S1 = DT("a2", 2 * F)
                nc.vector.tensor_tensor(
                    S1[:].rearrange("p (f c) -> p f c", c=2),
                    d0v[:, :, 0:2], d0v[:, :, 2:4], ALU.add)
                s1v = S1[:].rearrange("p (f c) -> p f c", c=2)
                nc.vector.tensor_tensor(
                    RC[:, 3 * F : 4 * F], s1v[:, :, 0], s1v[:, :, 1], ALU.add)
                rx = RC[:, 0:F]
                ry = RC[:, F : 2 * F]
                rz = RC[:, 2 * F : 3 * F]
                rw = RC[:, 3 * F : 4 * F]

                # ---- stage B ----
                z1 = WT("z1", bufs=2, dt=BF16)
                z2 = WT("z2", bufs=2, dt=BF16)
                nc.scalar.activation(z1[:], rx[:], AF.Square)
                nc.scalar.activation(z2[:], ry[:], AF.Square)
                # q = vn2 (bf16: 621ns adds vs 1100 f32)
                q = WT("q", bufs=2, dt=BF16)
                nc.vector.tensor_tensor(q[:], z1[:], z2[:], ALU.add)
                z3 = WT("z1", bufs=2, dt=BF16)
                nc.scalar.activation(z3[:], rz[:], AF.Square)
                nc.vector.tensor_tensor(q[:], q[:], z3[:], ALU.add)  # q = vn2
                # dn2 chain in bf16, then f32 from Sqrt onward
                w2 = WT("w2", bufs=2, dt=BF16)
                nc.scalar.activation(w2[:], rw[:], AF.Square)  # rw^2
                nc.vector.tensor_tensor(w2[:], w2[:], q[:], ALU.add)  # dn2
                w = WT("w", bufs=2)
                nc.scalar.activation(w[:], w2[:], AF.Sqrt)  # dn (f32)
                # den = |rw| + dn
                wab = WT("wab", bufs=2)
                nc.scalar.activation(wab[:], rw[:], AF.Abs)
                nc.vector.tensor_tensor(w[:], wab[:], w[:], ALU.add)
                winv = WT("winv", bufs=2)
                nc.vector.reciprocal_approx_fast(winv[:], w[:])  # 1/den
                vn = WT("vn", bufs=2)
                nc.scalar.activation(vn[:], q[:], AF.Sqrt)  # vn (f32)
                nc.vector.tensor_tensor(winv[:], vn[:], winv[:], ALU.mult)  # r
                nc.scalar.activation(winv[:], winv[:], AF.Arctan)  # at

                junk = WT("junk", bufs=2, dt=BF16)
                nc.scalar.activation(
                    junk[:], winv[:], AF.Square, scale=_SQ8,
                    accum_out=t1cols[:, i : i + 1],
                )

                ivn = WT("ivn", bufs=2)
                nc.vector.reciprocal_approx_fast(ivn[:], vn[:])  # 1/vn
                # g = 4*at/vn (f32)
                nc.vector.scalar_tensor_tensor(
                    ivn[:], winv[:], 4.0, ivn[:], ALU.mult, ALU.mult
                )

                for k, rv in enumerate((rx, ry, rz)):
                    av = WT("av", bufs=2, dt=BF16)
                    nc.scalar.activation(av[:], rv[:], AF.Abs)
                    u = WT("u", bufs=2, dt=BF16)
                    nc.vector.tensor_tensor(u[:], av[:], ivn[:], ALU.mult)
                    # h = relu(u - 1), then accumulate h^2 along the free dim
                    hr = WT("hr", bufs=2, dt=BF16)
                    nc.scalar.activation(hr[:], u[:], AF.Relu, bias=negone[:])
                    junk2 = WT("junk", bufs=2, dt=BF16)
                    nc.scalar.activation(
                        junk2[:], hr[:], AF.Square,
                        accum_out=hacols[:, 3 * i + k : 3 * i + k + 1],
                    )

            res = acc_pool.tile([P, 2], F32, tag="res", name="res")
            nc.vector.tensor_reduce(
                res[:, 0:1], t1cols[:], mybir.AxisListType.X, ALU.add
            )
            nc.vector.tensor_reduce(
                res[:, 1:2], hacols[:], mybir.AxisListType.X, ALU.add
            )
            nc.gpsimd.dma_start(out.ap(), res[:])

    nc.compile()
    return nc


_CACHED_NC = None


def _get_nc():
    global _CACHED_NC
    if _CACHED_NC is None:
        _CACHED_NC = _build_nc()
    return _CACHED_NC


def run_sharded(tq_full, pq_full, **run_kwargs):
    """Run the SPMD kernel; returns BassKernelResults."""
    nc = _get_nc()
    in_maps = []
    for c in range(NCORES):
        sl = slice(c * S, (c + 1) * S)
        in_maps.append(
            {
                "tq": np.ascontiguousarray(tq_full[sl]),
                "pq": np.ascontiguousarray(pq_full[sl]),
            }
        )
    return run_bass_kernel_spmd(nc, in_maps, list(range(NCORES)), **run_kwargs)


def kernel(
    true_quaternions: np.ndarray,
    predicted_quaternion: np.ndarray,
    batch_X: np.ndarray = None,
    **_ignored,
) -> np.ndarray:
    res = run_sharded(true_quaternions, predicted_quaternion)
    total = 0.0
    for core in res.results:
        v = core["out"].astype(np.float64)
        total += v[:, 0].sum() - 0.5 * v[:, 1].sum()
    loss = total / (3.0 * B)
    return np.float32(loss)



# revision 5
# speedup vs baseline: 1.3953x; 1.0392x over previous
"""Trainium2 Bass kernel for nn_CustomQuaternionLoss.

Computes mean over B samples of mean-over-3-components HuberLoss(delta=1)
of the rotation vector of  R(pred_quat) * R(true_quat)^-1.

Mathematical reformulation (verified vs reference, rel err ~2e-7):
  d = p (x) conj(t)   (unnormalized quaternion product; norms divide out)
  Every downstream quantity is even in each component of d, so the
  w>=0 canonicalization is free (|rw|) and component signs are dropped.
  angle = 2*atan2(|v|, |w|)  = 4*atan(|v| / (sqrt(|v|^2+w^2) + |w|))
      (tangent half-angle form keeps the Arctan spline argument in [0,1];
       the ACT Arctan table is only valid on [-pi/2, pi/2])
  rotvec_i = angle * v_i / |v|
  huber(x) = 0.5*x^2 - 0.5*relu(|x|-1)^2
  sum_i huber(rotvec_i) = 0.5*angle^2 - 0.5*sum_i relu(angle*|v_i|/|v| - 1)^2
  (the small-angle Taylor branch of the reference is unreachable for the
   randn inputs: min angle ~0.012 >> 1e-3)

Sharding: batch dim split evenly across 8 cores; each core reduces its
524288 samples to a per-partition [128,2] partial-sum pair; host combines.
batch_X is unused by the reference and is never touched.

Built on Bacc (not raw Bass): walrus only accepts one sync wait per
instruction, and Bacc's generate_event_semaphores pass splits multi-wait
instructions into EventSemaphore + op.
"""

import numpy as np

import concourse.bacc as bacc
import concourse.mybir as mybir
import concourse.tile as tile
from concourse.bass_utils import run_bass_kernel_spmd

B = 4194304
NCORES = 8
S = B // NCORES  # samples per core
P = 128
F = 1024  # samples per partition per tile
NT = S // (P * F)  # tiles per core (4)

F32 = mybir.dt.float32
BF16 = mybir.dt.bfloat16
AF = mybir.ActivationFunctionType
ALU = mybir.AluOpType

_SQ8 = float(np.sqrt(8.0))  # 0.5*angle^2 = 0.5*(4*at)^2 = (sqrt(8)*at)^2


def _build_nc():
    nc = bacc.Bacc(
        "TRN2", target_bir_lowering=False, debug=False, num_devices=NCORES
    )
    tq = nc.dram_tensor("tq", [S, 4], F32, kind="ExternalInput")
    pq = nc.dram_tensor("pq", [S, 4], F32, kind="ExternalInput")
    out = nc.dram_tensor("out", [P, 2], F32, kind="ExternalOutput")

    tqv = tq.ap().rearrange("(n p f) c -> n p (f c)", p=P, f=F)
    pqv = pq.ap().rearrange("(n p f) c -> n p (f c)", p=P, f=F)

    with tile.TileContext(nc) as tc:
        with (
            tc.tile_pool(name="io", bufs=2) as io_pool,
            tc.tile_pool(name="wk", bufs=1) as wk,
            tc.tile_pool(name="acc", bufs=1) as acc_pool,
        ):
            t1cols = acc_pool.tile([P, NT], F32, tag="t1c", name="t1cols")
            hacols = acc_pool.tile([P, 3 * NT], F32, tag="hac", name="hacols")
            negone = acc_pool.tile([P, 1], F32, tag="negone", name="negone")
            nc.vector.memset(negone[:], -1.0)

            def WT(tag, bufs=None, dt=F32):
                return wk.tile([P, F], dt, tag=tag, name=tag, bufs=bufs)

            def WT4(tag):
                return wk.tile([P, 4 * F], BF16, tag=tag, name=tag, bufs=2)

            for i in range(NT):
                tt = io_pool.tile([P, 4 * F], F32, tag="tq", name="tqt")
                pt = io_pool.tile([P, 4 * F], F32, tag="pq", name="pqt")
                nc.sync.dma_start(tt[:], tqv[i])
                nc.sync.dma_start(pt[:], pqv[i])

                # bf16 copies: strided product reads then run at 2B width
                # (measured 1211ns vs 1952ns per [P,1024] stride-4 mult)
                tb = wk.tile([P, 4 * F], BF16, tag="tb", name="tb", bufs=1)
                pb = wk.tile([P, 4 * F], BF16, tag="pb", name="pb", bufs=1)
                nc.vector.tensor_copy(tb[:], tt[:])
                nc.scalar.activation(pb[:], pt[:], AF.Copy)

                tv = tb[:].rearrange("p (f c) -> p c f", c=4)
                pv = pb[:].rearrange("p (f c) -> p c f", c=4)
                tx, ty, tz, tw = (tv[:, c, :] for c in range(4))
                px, py, pz, pw = (pv[:, c, :] for c in range(4))

                # ---- stage A: d = p (x) conj(t), diagonal product scheme ----
                # Products grouped by component-diagonal (t_i*p_{i+k}) so one
                # TT op covers 2-4 products via packed [F,k] access patterns:
                #   rx = (t3p0-t0p3) - (t2p1-t1p2)
                #   ry = (t2p0-t0p2) + (t3p1-t1p3)
                #   rz = (t3p2-t2p3) - (t1p0-t0p1)
                #   rw = t0p0+t1p1+t2p2+t3p3
                # (per-component global signs are free: downstream is even)
                tv4 = tb[:].rearrange("p (f c) -> p f c", c=4)
                pv4 = pb[:].rearrange("p (f c) -> p f c", c=4)

                def DT(tag, w):
                    return wk.tile([P, w], BF16, tag=tag, name=tag, bufs=1)

                D0 = DT("d0", 4 * F)
                nc.vector.tensor_tensor(D0[:], tb[:], pb[:], ALU.mult)
                A1 = DT("a1", 3 * F)
                nc.vector.tensor_tensor(
                    A1[:].rearrange("p (f c) -> p f c", c=3),
                    tv4[:, :, 0:3], pv4[:, :, 1:4], ALU.mult)
                B1 = DT("b1", 3 * F)
                nc.vector.tensor_tensor(
                    B1[:].rearrange("p (f c) -> p f c", c=3),
                    tv4[:, :, 1:4], pv4[:, :, 0:3], ALU.mult)
                A2 = DT("a2", 2 * F)
                nc.vector.tensor_tensor(
                    A2[:].rearrange("p (f c) -> p f c", c=2),
                    tv4[:, :, 0:2], pv4[:, :, 2:4], ALU.mult)
                B2 = DT("b2", 2 * F)
                nc.vector.tensor_tensor(
                    B2[:].rearrange("p (f c) -> p f c", c=2),
                    tv4[:, :, 2:4], pv4[:, :, 0:2], ALU.mult)
                A3 = DT("a3", F)
                nc.vector.tensor_tensor(A3[:], tv4[:, :, 0], pv4[:, :, 3], ALU.mult)
                B3 = DT("b3", F)
                nc.vector.tensor_tensor(B3[:], tv4[:, :, 3], pv4[:, :, 0], ALU.mult)

                # combines (in-place diffs, then slot compactions into RC)
                nc.vector.tensor_tensor(B1[:], B1[:], A1[:], ALU.subtract)
                nc.vector.tensor_tensor(B2[:], B2[:], A2[:], ALU.subtract)
                nc.vector.tensor_tensor(B3[:], B3[:], A3[:], ALU.subtract)
                RC = WT4("rc4")
                cb1v = B1[:].rearrange("p (f c) -> p f c", c=3)
                cb2v = B2[:].rearrange("p (f c) -> p f c", c=2)
                nc.vector.tensor_tensor(
                    RC[:, 0:F], B3[:], cb1v[:, :, 1], ALU.subtract)
                nc.vector.tensor_tensor(
                    RC[:, F : 2 * F], cb2v[:, :, 0], cb2v[:, :, 1], ALU.add)
                nc.vector.tensor_tensor(
                    RC[:, 2 * F : 3 * F], cb1v[:, :, 2], cb1v[:, :, 0],
                    ALU.subtract)
                d0v = D0[:].rearrange("p (f c) -> p f c", c=4)
                S1 = DT("a2", 2 * F)
                nc.vector.tensor_tensor(
                    S1[:].rearrange("p (f c) -> p f c", c=2),
                    d0v[:, :, 0:2], d0v[:, :, 2:4], ALU.add)
                s1v = S1[:].rearrange("p (f c) -> p f c", c=2)
                nc.vector.tensor_tensor(
                    RC[:, 3 * F : 4 * F], s1v[:, :, 0], s1v[:, :, 1], ALU.add)
                rx = RC[:, 0:F]
                ry = RC[:, F : 2 * F]
                rz = RC[:, 2 * F : 3 * F]
                rw = RC[:, 3 * F : 4 * F]

                # ---- stage B ----
                z1 = WT("z1", bufs=2, dt=BF16)
                z2 = WT("z2", bufs=2, dt=BF16)
                nc.scalar.activation(z1[:], rx[:], AF.Square)
                nc.scalar.activation(z2[:], ry[:], AF.Square)
                # q = vn2 (bf16: 621ns adds vs 1100 f32)
                q = WT("q", bufs=2, dt=BF16)
                nc.vector.tensor_tensor(q[:], z1[:], z2[:], ALU.add)
                z3 = WT("z1", bufs=2, dt=BF16)
                nc.scalar.activation(z3[:], rz[:], AF.Square)
                nc.vector.tensor_tensor(q[:], q[:], z3[:], ALU.add)  # q = vn2
                # dn2 chain in bf16, then f32 from Sqrt onward
                w2 = WT("w2", bufs=2, dt=BF16)
                nc.scalar.activation(w2[:], rw[:], AF.Square)  # rw^2
                nc.vector.tensor_tensor(w2[:], w2[:], q[:], ALU.add)  # dn2
                w = WT("w", bufs=2)
                nc.scalar.activation(w[:], w2[:], AF.Sqrt)  # dn (f32)
                # den = |rw| + dn
                wab = WT("wab", bufs=2)
                nc.scalar.activation(wab[:], rw[:], AF.Abs)
                nc.vector.tensor_tensor(w[:], wab[:], w[:], ALU.add)
                winv = WT("winv", bufs=2)
                nc.vector.reciprocal_approx_fast(winv[:], w[:])  # 1/den
                vn = WT("vn", bufs=2)
                nc.scalar.activation(vn[:], q[:], AF.Sqrt)  # vn (f32)
                nc.vector.tensor_tensor(winv[:], vn[:], winv[:], ALU.mult)  # r
                nc.scalar.activation(winv[:], winv[:], AF.Arctan)  # at

                junk = WT("junk", bufs=2, dt=BF16)
                nc.scalar.activation(
                    junk[:], winv[:], AF.Square, scale=_SQ8,
                    accum_out=t1cols[:, i : i + 1],
                )

                ivn = WT("ivn", bufs=2)
                nc.scalar.activation(ivn[:], q[:], AF.Abs_reciprocal_sqrt)
                # g = at/vn -> bf16 (the *4 is folded into av's Abs scale)
                gb = WT("gb", bufs=2, dt=BF16)
                nc.vector.tensor_tensor(gb[:], winv[:], ivn[:], ALU.mult)

                for k, rv in enumerate((rx, ry, rz)):
                    av = WT("av", bufs=2, dt=BF16)
                    nc.scalar.activation(av[:], rv[:], AF.Abs, scale=4.0)
                    u = WT("u", bufs=2, dt=BF16)
                    nc.vector.tensor_tensor(u[:], av[:], gb[:], ALU.mult)
                    # h = relu(u - 1), then accumulate h^2 along the free dim
                    hr = WT("hr", bufs=2, dt=BF16)
                    nc.scalar.activation(hr[:], u[:], AF.Relu, bias=negone[:])
                    junk2 = WT("junk", bufs=2, dt=BF16)
                    nc.scalar.activation(
                        junk2[:], hr[:], AF.Square,
                        accum_out=hacols[:, 3 * i + k : 3 * i + k + 1],
                    )

            res = acc_pool.tile([P, 2], F32, tag="res", name="res")
            nc.vector.tensor_reduce(
                res[:, 0:1], t1cols[:], mybir.AxisListType.X, ALU.add
            )
            nc.vector.tensor_reduce(
                res[:, 1:2], hacols[:], mybir.AxisListType.X, ALU.add
            )
            nc.gpsimd.dma_start(out.ap(), res[:])

    nc.compile()
    return nc


_CACHED_NC = None


def _get_nc():
    global _CACHED_NC
    if _CACHED_NC is None:
        _CACHED_NC = _build_nc()
    return _CACHED_NC


def run_sharded(tq_full, pq_full, **run_kwargs):
    """Run the SPMD kernel; returns BassKernelResults."""
    nc = _get_nc()
    in_maps = []
    for c in range(NCORES):
        sl = slice(c * S, (c + 1) * S)
        in_maps.append(
            {
                "tq": np.ascontiguousarray(tq_full[sl]),
                "pq": np.ascontiguousarray(pq_full[sl]),
            }
        )
    return run_bass_kernel_spmd(nc, in_maps, list(range(NCORES)), **run_kwargs)


def kernel(
    true_quaternions: np.ndarray,
    predicted_quaternion: np.ndarray,
    batch_X: np.ndarray = None,
    **_ignored,
) -> np.ndarray:
    res = run_sharded(true_quaternions, predicted_quaternion)
    total = 0.0
    for core in res.results:
        v = core["out"].astype(np.float64)
        total += v[:, 0].sum() - 0.5 * v[:, 1].sum()
    loss = total / (3.0 * B)
    return np.float32(loss)

